# revision 1
# baseline (speedup 1.0000x reference)
"""Trainium2 Bass kernel for nn_LinearLoopLayer: out = x @ weight.T + bias.

x: (2048, 4096) f32, weight: (4096, 4096) f32, bias: (4096,) f32.
Sharding: 2 batch-halves x 4 out-feature-quarters across 8 NeuronCores.
Each core computes outT_shard[j, b] = sum_i wT[i, j] * xT[i, b] + bias[j]
with host-pre-transposed xT/wT so the contraction dim i is the SBUF
partition dim (no on-device transposes).

Matmuls run in float32r (full-rate PE mode for 4-byte floats, ~1e-4 rel
err vs ~3e-7 for plain float32 at 1/4 rate; flip with LINEAR_MM_DT=f32).

DMA ring use: xt + out on the sync (SP) HWDGE ring, wt on the scalar
(ACT) ring, so pass-1 weight tiles aren't queued behind the 16MB xt
stream (HWDGE is FIFO per issuing engine). xt loads are interleaved
into the pass-1 compute loop to keep ring order = consumption order.
"""

import os
import sys

import numpy as np

sys.path.insert(0, "/opt/trn_rl_repo")

import concourse.mybir as mybir
from concourse import bacc, tile
from concourse.bass_utils import run_bass_kernel_spmd

P = 128
B, K, J = 2048, 4096, 4096
NCORES = 8
B_SPLIT, J_SPLIT = 2, 4
BL, JL = B // B_SPLIT, J // J_SPLIT  # per-core local batch / out-features
KT = K // P  # contraction tiles
NB = BL // 512  # moving-dim (batch) blocks per core
JS = JL // 512  # j-super blocks (512 features) per core
JSUB = 512 // P  # 128-feature psum row-blocks per j-super

_DT_BY_NAME = {
    "f32": mybir.dt.float32,
    "f32r": mybir.dt.float32r,
    "bf16": mybir.dt.bfloat16,
}
_MM_DT_NAME = os.environ.get("LINEAR_MM_DT", "f32r")


def _build(mm_dt):
    """One SPMD program; per-core differences come only via input shards."""
    nc = bacc.Bacc(None, target_bir_lowering=False)
    xt = nc.declare_dram_parameter("xt", [K, BL], mm_dt, isOutput=False)
    wt = nc.declare_dram_parameter("wt", [K, JL], mm_dt, isOutput=False)
    biasT = nc.declare_dram_parameter(
        "biasT", [P, JL // P], mybir.dt.float32, isOutput=False
    )
    out = nc.declare_dram_parameter("out", [JL, BL], mybir.dt.float32, isOutput=True)

    f32 = mybir.dt.float32
    with tile.TileContext(nc) as tc:
        with (
            tc.tile_pool(name="xtp", bufs=KT) as xt_pool,
            tc.tile_pool(name="wtp", bufs=6) as wt_pool,
            tc.tile_pool(name="outp", bufs=4) as out_pool,
            tc.tile_pool(name="biasp", bufs=1) as bias_pool,
            tc.tile_pool(name="psum", bufs=8, space="PSUM") as psum_pool,
        ):
            bias_sb = bias_pool.tile([P, JL // P], f32)
            nc.sync.dma_start(bias_sb[:], biasT[:, :])

            xt_tiles = [None] * KT
            for js in range(JS):
                ps = [
                    [psum_pool.tile([P, 512], f32, name="ps") for bb in range(NB)]
                    for jsub in range(JSUB)
                ]
                for i in range(KT):
                    if js == 0:
                        # xt shard stays resident; loaded in consumption order
                        t = xt_pool.tile([P, BL], mm_dt, name="xt")
                        nc.sync.dma_start(t[:], xt[i * P : (i + 1) * P, :])
                        xt_tiles[i] = t
                    wt_t = wt_pool.tile([P, 512], mm_dt, name="wt")
                    nc.scalar.dma_start(
                        wt_t[:], wt[i * P : (i + 1) * P, js * 512 : (js + 1) * 512]
                    )
                    for jsub in range(JSUB):
                        for bb in range(NB):
                            nc.tensor.matmul(
                                ps[jsub][bb][:],
                                wt_t[:, jsub * P : (jsub + 1) * P],
                                xt_tiles[i][:, bb * 512 : (bb + 1) * 512],
                                start=(i == 0),
                                stop=(i == KT - 1),
                            )
                for jsub in range(JSUB):
                    jb = js * JSUB + jsub
                    for bb in range(NB):
                        o = out_pool.tile([P, 512], f32, name="o")
                        nc.vector.tensor_scalar_add(
                            o[:], ps[jsub][bb][:], bias_sb[:, jb : jb + 1]
                        )
                        nc.sync.dma_start(
                            out[jb * P : (jb + 1) * P, bb * 512 : (bb + 1) * 512], o[:]
                        )
    nc.finalize()
    return nc


_NC_CACHE = {}


def _get_nc(mm_dt_name):
    if mm_dt_name not in _NC_CACHE:
        _NC_CACHE[mm_dt_name] = _build(_DT_BY_NAME[mm_dt_name])
    return _NC_CACHE[mm_dt_name]


def _make_in_maps(x, weight, bias):
    x = np.asarray(x, dtype=np.float32)
    if x.ndim == 4:
        x = x.reshape(x.shape[0], -1)
    weight = np.asarray(weight, dtype=np.float32)
    bias = np.asarray(bias, dtype=np.float32)
    in_maps = []
    for c in range(NCORES):
        bh, jq = divmod(c, J_SPLIT)
        bq = bias[jq * JL : (jq + 1) * JL]
        in_maps.append(
            {
                "xt": np.ascontiguousarray(x[bh * BL : (bh + 1) * BL].T),
                "wt": np.ascontiguousarray(weight[jq * JL : (jq + 1) * JL].T),
                "biasT": np.ascontiguousarray(bq.reshape(JL // P, P).T),
            }
        )
    return in_maps


def _assemble(results):
    out = np.empty((B, J), dtype=np.float32)
    for c in range(NCORES):
        bh, jq = divmod(c, J_SPLIT)
        out[bh * BL : (bh + 1) * BL, jq * JL : (jq + 1) * JL] = results[c]["out"].T
    return out


def run(x, weight, bias, mm_dt_name=None, trace=False, **kwargs):
    nc = _get_nc(mm_dt_name or _MM_DT_NAME)
    in_maps = _make_in_maps(x, weight, bias)
    res = run_bass_kernel_spmd(
        nc, in_maps, core_ids=list(range(NCORES)), trace=trace, **kwargs
    )
    return _assemble(res.results), res


def kernel(x, weight, bias):
    out, _ = run(x, weight, bias)
    return out



# revision 6
# speedup vs baseline: 13.4756x; 13.4756x over previous
"""Trainium2 Bass kernel for nn_LinearLoopLayer: out = x @ weight.T + bias.

x: (2048, 4096) f32, weight: (4096, 4096) f32, bias: (4096,) f32.
Sharding: 2 batch-halves x 4 out-feature-quarters across 8 NeuronCores.
Each core computes outT_shard[j, b] = sum_i wT[i, j] * xT[i, b] + bias[j]
with host-pre-transposed xT/wT so the contraction dim i is the SBUF
partition dim (no on-device transposes).

Matmuls run in float32r (full-rate PE mode for 4-byte floats, ~1e-4 rel
err vs ~3e-7 for plain float32 at 1/4 rate; flip with LINEAR_MM_DT=f32).

DMA ring use: xt + out on the sync (SP) HWDGE ring, wt on the scalar
(ACT) ring, so pass-1 weight tiles aren't queued behind the 16MB xt
stream (HWDGE is FIFO per issuing engine). xt loads are interleaved
into the pass-1 compute loop to keep ring order = consumption order.
"""

import os
import sys

import numpy as np

sys.path.insert(0, "/opt/trn_rl_repo")

import concourse.mybir as mybir
from concourse import bacc, tile
from concourse.bass_utils import run_bass_kernel_spmd

P = 128
B, K, J = 2048, 4096, 4096
NCORES = 8
B_SPLIT, J_SPLIT = 2, 4
BL, JL = B // B_SPLIT, J // J_SPLIT  # per-core local batch / out-features
KT = K // P  # contraction tiles
NB = BL // 512  # moving-dim (batch) blocks per core
JS = JL // 512  # j-super blocks (512 features) per core
JSUB = 512 // P  # 128-feature psum row-blocks per j-super

_DT_BY_NAME = {
    "f32": mybir.dt.float32,
    "f32r": mybir.dt.float32r,
    "bf16": mybir.dt.bfloat16,
}
_MM_DT_NAME = os.environ.get("LINEAR_MM_DT", "f32r")


def _build(mm_dt, nrep=1):
    """One SPMD program; per-core differences come only via input shards.

    nrep > 1 wraps the whole body in a hardware For loop — benchmarking
    only (amortizes the ~73ms axon per-call RPC overhead so per-rep HW
    time can be extracted by differencing two rep counts).
    """
    nc = bacc.Bacc(None, target_bir_lowering=False)
    xt = nc.declare_dram_parameter("xt", [K, BL], mm_dt, isOutput=False)
    wt = nc.declare_dram_parameter("wt", [K, JL], mm_dt, isOutput=False)
    biasT = nc.declare_dram_parameter(
        "biasT", [P, JL // P], mybir.dt.float32, isOutput=False
    )
    out = nc.declare_dram_parameter("out", [JL, BL], mybir.dt.float32, isOutput=True)

    f32 = mybir.dt.float32
    with tile.TileContext(nc) as tc:
        with (
            tc.tile_pool(name="xtp", bufs=KT) as xt_pool,
            tc.tile_pool(name="wtp", bufs=6) as wt_pool,
            tc.tile_pool(name="outp", bufs=4) as out_pool,
            tc.tile_pool(name="biasp", bufs=1) as bias_pool,
            tc.tile_pool(name="psum", bufs=8, space="PSUM") as psum_pool,
        ):
            from contextlib import nullcontext

            rep_ctx = (
                tc.For_i(0, nrep, 1, hint_engines=(mybir.EngineType.PE,))
                if nrep > 1
                else nullcontext()
            )
            with rep_ctx:
                bias_sb = bias_pool.tile([P, JL // P], f32)
                nc.sync.dma_start(bias_sb[:], biasT[:, :])

                xt_tiles = [None] * KT
                for js in range(JS):
                    ps = [
                        [psum_pool.tile([P, 512], f32, name="ps") for bb in range(NB)]
                        for jsub in range(JSUB)
                    ]
                    for i in range(KT):
                        if js == 0:
                            # xt shard stays resident; loaded in consumption order
                            t = xt_pool.tile([P, BL], mm_dt, name="xt")
                            nc.sync.dma_start(t[:], xt[i * P : (i + 1) * P, :])
                            xt_tiles[i] = t
                        wt_t = wt_pool.tile([P, 512], mm_dt, name="wt")
                        nc.scalar.dma_start(
                            wt_t[:], wt[i * P : (i + 1) * P, js * 512 : (js + 1) * 512]
                        )
                        for jsub in range(JSUB):
                            for bb in range(NB):
                                nc.tensor.matmul(
                                    ps[jsub][bb][:],
                                    wt_t[:, jsub * P : (jsub + 1) * P],
                                    xt_tiles[i][:, bb * 512 : (bb + 1) * 512],
                                    start=(i == 0),
                                    stop=(i == KT - 1),
                                )
                    for jsub in range(JSUB):
                        jb = js * JSUB + jsub
                        for bb in range(NB):
                            o = out_pool.tile([P, 512], f32, name="o")
                            nc.vector.tensor_scalar_add(
                                o[:], ps[jsub][bb][:], bias_sb[:, jb : jb + 1]
                            )
                            nc.sync.dma_start(
                                out[jb * P : (jb + 1) * P, bb * 512 : (bb + 1) * 512],
                                o[:],
                            )
    nc.finalize()
    return nc


_NC_CACHE = {}


def _get_nc(mm_dt_name, nrep=1):
    key = (mm_dt_name, nrep)
    if key not in _NC_CACHE:
        _NC_CACHE[key] = _build(_DT_BY_NAME[mm_dt_name], nrep=nrep)
    return _NC_CACHE[key]


def _make_in_maps(x, weight, bias, mm_dt):
    np_dt = mybir.dt.np(mm_dt)
    x = np.asarray(x, dtype=np.float32)
    if x.ndim == 4:
        x = x.reshape(x.shape[0], -1)
    weight = np.asarray(weight, dtype=np.float32)
    bias = np.asarray(bias, dtype=np.float32)
    xT = np.ascontiguousarray(x.T.astype(np_dt))  # [K, B]
    wT = np.ascontiguousarray(weight.T.astype(np_dt))  # [K, J]
    in_maps = []
    for c in range(NCORES):
        bh, jq = divmod(c, J_SPLIT)
        bq = bias[jq * JL : (jq + 1) * JL]
        in_maps.append(
            {
                "xt": np.ascontiguousarray(xT[:, bh * BL : (bh + 1) * BL]),
                "wt": np.ascontiguousarray(wT[:, jq * JL : (jq + 1) * JL]),
                "biasT": np.ascontiguousarray(bq.reshape(JL // P, P).T),
            }
        )
    return in_maps


def _assemble(results):
    out = np.empty((B, J), dtype=np.float32)
    for c in range(NCORES):
        bh, jq = divmod(c, J_SPLIT)
        out[bh * BL : (bh + 1) * BL, jq * JL : (jq + 1) * JL] = results[c]["out"].T
    return out


def run(x, weight, bias, mm_dt_name=None, trace=False, nrep=1, **kwargs):
    mm_dt_name = mm_dt_name or _MM_DT_NAME
    nc = _get_nc(mm_dt_name, nrep=nrep)
    in_maps = _make_in_maps(x, weight, bias, _DT_BY_NAME[mm_dt_name])
    res = run_bass_kernel_spmd(
        nc, in_maps, core_ids=list(range(NCORES)), trace=trace, **kwargs
    )
    return _assemble(res.results), res


def kernel(x, weight, bias):
    out, _ = run(x, weight, bias)
    return out



# revision 11
# speedup vs baseline: 478.8508x; 35.5346x over previous
"""Trainium2 Bass kernel for nn_LinearLoopLayer: out = x @ weight.T + bias.

x: (2048, 4096) f32, weight: (4096, 4096) f32, bias: (4096,) f32.
Sharding: 2 batch-halves x 4 out-feature-quarters across 8 NeuronCores.
Each core computes outT_shard[j, b] = sum_i wT[i, j] * xT[i, b] + bias[j]
with host-pre-transposed xT/wT so the contraction dim i is the SBUF
partition dim (no on-device transposes).

Matmuls run in float32r (full-rate PE mode for 4-byte floats, ~1e-4 rel
err vs ~3e-7 for plain float32 at 1/4 rate; flip with LINEAR_MM_DT=f32).

DMA ring use: xt + out on the sync (SP) HWDGE ring, wt on the scalar
(ACT) ring, so pass-1 weight tiles aren't queued behind the 16MB xt
stream (HWDGE is FIFO per issuing engine). xt loads are interleaved
into the pass-1 compute loop to keep ring order = consumption order.
"""

import os
import sys

import numpy as np

sys.path.insert(0, "/opt/trn_rl_repo")

import concourse.mybir as mybir
from concourse import bacc, tile
from concourse.bass_utils import run_bass_kernel_spmd

P = 128
B, K, J = 2048, 4096, 4096
NCORES = 8
B_SPLIT, J_SPLIT = 2, 4
BL, JL = B // B_SPLIT, J // J_SPLIT  # per-core local batch / out-features
KT = K // P  # contraction tiles
JS = JL // 512  # j-super blocks (512 features) per core
JSUB = 512 // P  # 128-feature psum row-blocks per j-super

_DT_BY_NAME = {
    "f32": mybir.dt.float32,
    "f32r": mybir.dt.float32r,
    "bf16": mybir.dt.bfloat16,
}
_MM_DT_NAME = os.environ.get("LINEAR_MM_DT", "f32r")


def _build(mm_dt, nrep=1):
    """One SPMD program; per-core differences come only via input shards.

    nrep > 1 wraps the whole body in a hardware For loop — benchmarking
    only (amortizes the ~73ms axon per-call RPC overhead so per-rep HW
    time can be extracted by differencing two rep counts).
    """
    nc = bacc.Bacc(None, target_bir_lowering=False)
    xt = nc.declare_dram_parameter("xt", [K, BL], mm_dt, isOutput=False)
    wt = nc.declare_dram_parameter("wt", [K, JL], mm_dt, isOutput=False)
    biasT = nc.declare_dram_parameter(
        "biasT", [P, JL // P], mybir.dt.float32, isOutput=False
    )
    out = nc.declare_dram_parameter("out", [JL, BL], mybir.dt.float32, isOutput=True)

    f32 = mybir.dt.float32
    # moving-dim (batch) block: FP32 matmul caps at 512 rows, 16-bit at 1024
    MV = 1024 if mm_dt == mybir.dt.bfloat16 else 512
    NB = BL // MV
    # PSUM is 8 banks x 2KB/partition; a [P, MV] f32 tile is MV*4 bytes/partition
    psum_bufs = (8 * 2048) // (MV * 4)
    with tile.TileContext(nc) as tc:
        with (
            tc.tile_pool(name="xtp", bufs=KT) as xt_pool,
            tc.tile_pool(name="wtp", bufs=6) as wt_pool,
            tc.tile_pool(name="outp", bufs=4) as out_pool,
            tc.tile_pool(name="biasp", bufs=1) as bias_pool,
            tc.tile_pool(name="psum", bufs=psum_bufs, space="PSUM") as psum_pool,
        ):
            from contextlib import nullcontext

            rep_ctx = (
                tc.For_i(0, nrep, 1, hint_engines=(mybir.EngineType.PE,))
                if nrep > 1
                else nullcontext()
            )
            with rep_ctx:
                bias_sb = bias_pool.tile([P, JL // P], f32)
                nc.sync.dma_start(bias_sb[:], biasT[:, :])

                xt_tiles = [None] * KT
                for js in range(JS):
                    ps = [
                        [psum_pool.tile([P, MV], f32, name="ps") for bb in range(NB)]
                        for jsub in range(JSUB)
                    ]
                    for i in range(KT):
                        if js == 0:
                            # xt shard stays resident; loaded in consumption order
                            t = xt_pool.tile([P, BL], mm_dt, name="xt")
                            nc.sync.dma_start(t[:], xt[i * P : (i + 1) * P, :])
                            xt_tiles[i] = t
                        wt_t = wt_pool.tile([P, 512], mm_dt, name="wt")
                        nc.scalar.dma_start(
                            wt_t[:], wt[i * P : (i + 1) * P, js * 512 : (js + 1) * 512]
                        )
                        for jsub in range(JSUB):
                            for bb in range(NB):
                                nc.tensor.matmul(
                                    ps[jsub][bb][:],
                                    wt_t[:, jsub * P : (jsub + 1) * P],
                                    xt_tiles[i][:, bb * MV : (bb + 1) * MV],
                                    start=(i == 0),
                                    stop=(i == KT - 1),
                                )
                    for jsub in range(JSUB):
                        jb = js * JSUB + jsub
                        for bb in range(NB):
                            o = out_pool.tile([P, MV], f32, name="o")
                            nc.vector.tensor_scalar_add(
                                o[:], ps[jsub][bb][:], bias_sb[:, jb : jb + 1]
                            )
                            nc.sync.dma_start(
                                out[jb * P : (jb + 1) * P, bb * MV : (bb + 1) * MV],
                                o[:],
                            )
    nc.finalize()
    return nc


_NC_CACHE = {}


def _get_nc(mm_dt_name, nrep=1):
    key = (mm_dt_name, nrep)
    if key not in _NC_CACHE:
        _NC_CACHE[key] = _build(_DT_BY_NAME[mm_dt_name], nrep=nrep)
    return _NC_CACHE[key]


def _make_in_maps(x, weight, bias, mm_dt):
    np_dt = mybir.dt.np(mm_dt)
    x = np.asarray(x, dtype=np.float32)
    if x.ndim == 4:
        x = x.reshape(x.shape[0], -1)
    weight = np.asarray(weight, dtype=np.float32)
    bias = np.asarray(bias, dtype=np.float32)
    xT = np.ascontiguousarray(x.T.astype(np_dt))  # [K, B]
    wT = np.ascontiguousarray(weight.T.astype(np_dt))  # [K, J]
    in_maps = []
    for c in range(NCORES):
        bh, jq = divmod(c, J_SPLIT)
        bq = bias[jq * JL : (jq + 1) * JL]
        in_maps.append(
            {
                "xt": np.ascontiguousarray(xT[:, bh * BL : (bh + 1) * BL]),
                "wt": np.ascontiguousarray(wT[:, jq * JL : (jq + 1) * JL]),
                "biasT": np.ascontiguousarray(bq.reshape(JL // P, P).T),
            }
        )
    return in_maps


def _assemble(results):
    out = np.empty((B, J), dtype=np.float32)
    for c in range(NCORES):
        bh, jq = divmod(c, J_SPLIT)
        out[bh * BL : (bh + 1) * BL, jq * JL : (jq + 1) * JL] = results[c]["out"].T
    return out


def run(x, weight, bias, mm_dt_name=None, trace=False, nrep=1, **kwargs):
    mm_dt_name = mm_dt_name or _MM_DT_NAME
    nc = _get_nc(mm_dt_name, nrep=nrep)
    in_maps = _make_in_maps(x, weight, bias, _DT_BY_NAME[mm_dt_name])
    res = run_bass_kernel_spmd(
        nc, in_maps, core_ids=list(range(NCORES)), trace=trace, **kwargs
    )
    return _assemble(res.results), res


def kernel(x, weight, bias):
    out, _ = run(x, weight, bias)
    return out



# revision 12
# speedup vs baseline: 487.7679x; 1.0186x over previous
"""Trainium2 Bass kernel for nn_LinearLoopLayer: out = x @ weight.T + bias.

x: (2048, 4096) f32, weight: (4096, 4096) f32, bias: (4096,) f32.
Sharding: 2 batch-halves x 4 out-feature-quarters across 8 NeuronCores.
Each core computes outT_shard[j, b] = sum_i wT[i, j] * xT[i, b] + bias[j]
with host-pre-transposed xT/wT so the contraction dim i is the SBUF
partition dim (no on-device transposes).

Matmuls run in float32r (full-rate PE mode for 4-byte floats, ~1e-4 rel
err vs ~3e-7 for plain float32 at 1/4 rate; flip with LINEAR_MM_DT=f32).

DMA ring use: xt + out on the sync (SP) HWDGE ring, wt on the scalar
(ACT) ring, so pass-1 weight tiles aren't queued behind the 16MB xt
stream (HWDGE is FIFO per issuing engine). xt loads are interleaved
into the pass-1 compute loop to keep ring order = consumption order.
"""

import os
import sys

import numpy as np

sys.path.insert(0, "/opt/trn_rl_repo")

import concourse.mybir as mybir
from concourse import bacc, tile
from concourse.bass_utils import run_bass_kernel_spmd

P = 128
B, K, J = 2048, 4096, 4096
NCORES = 8
B_SPLIT, J_SPLIT = 2, 4
BL, JL = B // B_SPLIT, J // J_SPLIT  # per-core local batch / out-features
KT = K // P  # contraction tiles
JS = JL // 512  # j-super blocks (512 features) per core
JSUB = 512 // P  # 128-feature psum row-blocks per j-super

_DT_BY_NAME = {
    "f32": mybir.dt.float32,
    "f32r": mybir.dt.float32r,
    "bf16": mybir.dt.bfloat16,
}
_MM_DT_NAME = os.environ.get("LINEAR_MM_DT", "f32r")


def _build(mm_dt, nrep=1):
    """One SPMD program; per-core differences come only via input shards.

    nrep > 1 wraps the whole body in a hardware For loop — benchmarking
    only (amortizes the ~73ms axon per-call RPC overhead so per-rep HW
    time can be extracted by differencing two rep counts).
    """
    nc = bacc.Bacc(None, target_bir_lowering=False)
    xt = nc.declare_dram_parameter("xt", [K, BL], mm_dt, isOutput=False)
    wt = nc.declare_dram_parameter("wt", [K, JL], mm_dt, isOutput=False)
    biasT = nc.declare_dram_parameter(
        "biasT", [P, JL // P], mybir.dt.float32, isOutput=False
    )
    out = nc.declare_dram_parameter("out", [JL, BL], mybir.dt.float32, isOutput=True)

    f32 = mybir.dt.float32
    # moving-dim (batch) block: one matmul's psum output is capped at one
    # 2KB bank = 512 f32 per partition (ISA s3d3_mm_num_elements)
    MV = 512
    NB = BL // MV
    # PSUM is 8 banks x 2KB/partition; a [P, MV] f32 tile is MV*4 bytes/partition
    psum_bufs = (8 * 2048) // (MV * 4)
    with tile.TileContext(nc) as tc:
        with (
            tc.tile_pool(name="xtp", bufs=KT) as xt_pool,
            tc.tile_pool(name="wtp", bufs=6) as wt_pool,
            tc.tile_pool(name="outp", bufs=4) as out_pool,
            tc.tile_pool(name="biasp", bufs=1) as bias_pool,
            tc.tile_pool(name="psum", bufs=psum_bufs, space="PSUM") as psum_pool,
        ):
            from contextlib import nullcontext

            rep_ctx = (
                tc.For_i(0, nrep, 1, hint_engines=(mybir.EngineType.PE,))
                if nrep > 1
                else nullcontext()
            )
            with rep_ctx:
                bias_sb = bias_pool.tile([P, JL // P], f32)
                nc.sync.dma_start(bias_sb[:], biasT[:, :])

                xt_tiles = [None] * KT
                for js in range(JS):
                    ps = [
                        [psum_pool.tile([P, MV], f32, name="ps") for bb in range(NB)]
                        for jsub in range(JSUB)
                    ]
                    for i in range(KT):
                        if js == 0:
                            # xt shard stays resident; loaded in consumption order
                            t = xt_pool.tile([P, BL], mm_dt, name="xt")
                            nc.sync.dma_start(t[:], xt[i * P : (i + 1) * P, :])
                            xt_tiles[i] = t
                        wt_t = wt_pool.tile([P, 512], mm_dt, name="wt")
                        nc.scalar.dma_start(
                            wt_t[:], wt[i * P : (i + 1) * P, js * 512 : (js + 1) * 512]
                        )
                        for jsub in range(JSUB):
                            for bb in range(NB):
                                nc.tensor.matmul(
                                    ps[jsub][bb][:],
                                    wt_t[:, jsub * P : (jsub + 1) * P],
                                    xt_tiles[i][:, bb * MV : (bb + 1) * MV],
                                    start=(i == 0),
                                    stop=(i == KT - 1),
                                )
                    for jsub in range(JSUB):
                        jb = js * JSUB + jsub
                        for bb in range(NB):
                            o = out_pool.tile([P, MV], f32, name="o")
                            nc.vector.tensor_scalar_add(
                                o[:], ps[jsub][bb][:], bias_sb[:, jb : jb + 1]
                            )
                            nc.sync.dma_start(
                                out[jb * P : (jb + 1) * P, bb * MV : (bb + 1) * MV],
                                o[:],
                            )
    nc.finalize()
    return nc


_NC_CACHE = {}


def _get_nc(mm_dt_name, nrep=1):
    key = (mm_dt_name, nrep)
    if key not in _NC_CACHE:
        _NC_CACHE[key] = _build(_DT_BY_NAME[mm_dt_name], nrep=nrep)
    return _NC_CACHE[key]


def _make_in_maps(x, weight, bias, mm_dt):
    np_dt = mybir.dt.np(mm_dt)
    x = np.asarray(x, dtype=np.float32)
    if x.ndim == 4:
        x = x.reshape(x.shape[0], -1)
    weight = np.asarray(weight, dtype=np.float32)
    bias = np.asarray(bias, dtype=np.float32)
    xT = np.ascontiguousarray(x.T.astype(np_dt))  # [K, B]
    wT = np.ascontiguousarray(weight.T.astype(np_dt))  # [K, J]
    in_maps = []
    for c in range(NCORES):
        bh, jq = divmod(c, J_SPLIT)
        bq = bias[jq * JL : (jq + 1) * JL]
        in_maps.append(
            {
                "xt": np.ascontiguousarray(xT[:, bh * BL : (bh + 1) * BL]),
                "wt": np.ascontiguousarray(wT[:, jq * JL : (jq + 1) * JL]),
                "biasT": np.ascontiguousarray(bq.reshape(JL // P, P).T),
            }
        )
    return in_maps


def _assemble(results):
    out = np.empty((B, J), dtype=np.float32)
    for c in range(NCORES):
        bh, jq = divmod(c, J_SPLIT)
        out[bh * BL : (bh + 1) * BL, jq * JL : (jq + 1) * JL] = results[c]["out"].T
    return out


def run(x, weight, bias, mm_dt_name=None, trace=False, nrep=1, **kwargs):
    mm_dt_name = mm_dt_name or _MM_DT_NAME
    nc = _get_nc(mm_dt_name, nrep=nrep)
    in_maps = _make_in_maps(x, weight, bias, _DT_BY_NAME[mm_dt_name])
    res = run_bass_kernel_spmd(
        nc, in_maps, core_ids=list(range(NCORES)), trace=trace, **kwargs
    )
    return _assemble(res.results), res


def kernel(x, weight, bias):
    out, _ = run(x, weight, bias)
    return out

